# revision 1
# baseline (speedup 1.0000x reference)
"""Trainium2 Bass kernel for DetCenterDense: shared 3x3 conv + ReLU + four
1x1 head convs (cls/box/dir/scr, sigmoid on scr), channel-concatenated output.

Full inputs in / full output out. Sharding: 8 cores = batch(4) x H-halves(2);
each core computes a [20, 256, 512] output shard from a [128, 258, 512]
haloed input shard. No inter-core communication.

Compute structure (per core): the 3x3 conv is 9 shifted 1x1 convs accumulated
in PSUM, processing output rows in pairs packed into one PSUM tile [128, 512]
(partitions 0:64 = row y, 64:128 = row y+1). Every matmul in the kernel is an
M=64 column-tile so the PE array stays in (128,64) mode throughout (no
mode-switch drains) and independent M=64 matmuls run two-per-slot:

  - "full" taps (middle input rows, packed [W_a|W_b] weights) issue as two
    concurrent col-tiled twins sharing one moving stream;
  - "edge" taps (rows a/d, M=64 each) pair up in one slot;
  - two consecutive pairs' head matmuls (M=40) share one slot.

That is 9.5 tensor-engine slots per row pair -- the matmul-count floor for
K=128/M<=128 -- and the measured steady state is ~216 ns/slot (N=512 bf16
issue-gap at 2.4 GHz) with <0.2% stall.

Activations stream as fp8e3 (E3M4; x pre-scaled by 2 with 1/2 folded into the
bf16 conv weights) to halve SBUF read traffic -- without this, concurrent
DVE/ACT/DMA traffic degrades the matmul issue rate ~20%. Conv weights stay
bf16 (fp8 weights would breach the 2e-2 accuracy gate; rel l2 is ~1.35e-2).
ReLU runs on DVE; head bias on DVE (identity rows) and ACT (sigmoid rows);
output staging flushes via DMAs spread across three queues.
"""

import numpy as np

HS = 256          # output rows per core shard
HALO = HS + 2     # input rows per core shard (1-row halo each side)
W = 512
CH = 8            # input rows per DMA chunk
NCHUNK = (HALO + CH - 1) // CH
XSCALE = 2.0      # fp8e3 pre-scale of x; 1/XSCALE folded into w_shared

_NC_CACHE = {}


def _build_nc():
    from contextlib import ExitStack

    import concourse.mybir as mybir
    import concourse.tile as tile
    from concourse import bacc

    f32 = mybir.dt.float32
    bf16 = mybir.dt.bfloat16
    f8e3 = mybir.dt.float8e3
    Sigmoid = mybir.ActivationFunctionType.Sigmoid
    Identity = mybir.ActivationFunctionType.Identity

    nc = bacc.Bacc("TRN2", target_bir_lowering=False, debug=False, num_devices=8)
    x_d = nc.dram_tensor("x", [128, HALO * W], f8e3, kind="ExternalInput").ap()
    wf1_d = nc.dram_tensor("wfull1", [128, 256], bf16, kind="ExternalInput").ap()
    wf2_d = nc.dram_tensor("wfull2", [128, 512], bf16, kind="ExternalInput").ap()
    wh_d = nc.dram_tensor("whalf", [128, 6 * 64], bf16, kind="ExternalInput").ap()
    whd_d = nc.dram_tensor("wheads", [128, 40], bf16, kind="ExternalInput").ap()
    b_d = nc.dram_tensor("b40", [40, 1], f32, kind="ExternalInput").ap()
    out_d = nc.dram_tensor("out", [20, HS * W], f32, kind="ExternalOutput").ap()

    with ExitStack() as ctx:
        tc = ctx.enter_context(tile.TileContext(nc))
        wpool = ctx.enter_context(tc.tile_pool(name="w", bufs=1))
        bfpool = ctx.enter_context(tc.tile_pool(name="xbf", bufs=6))
        xrpool = ctx.enter_context(tc.tile_pool(name="xr", bufs=4))
        opool = ctx.enter_context(tc.tile_pool(name="ot", bufs=3))
        ppool = ctx.enter_context(tc.tile_pool(name="pp", bufs=5, space="PSUM"))
        hpool = ctx.enter_context(tc.tile_pool(name="hp", bufs=2, space="PSUM"))

        # weights go on the scalar queue so their descriptor-issue cost does
        # not delay the x-chunk DMAs on the sync queue (first matmul latency)
        wf = wpool.tile([128, 6 * 128], bf16)
        # kx=1 blocks (first matmuls' weights) are a separate contiguous input
        # so their small DMA completes early and the first LDWEIGHTS releases
        nc.scalar.dma_start(wf[:, 0:256], wf1_d[:])
        nc.scalar.dma_start(wf[:, 256:768], wf2_d[:])
        wh = wpool.tile([128, 6 * 64], bf16)
        nc.scalar.dma_start(wh[:], wh_d[:])
        whd = wpool.tile([128, 40], bf16)
        nc.scalar.dma_start(whd[:], whd_d[:])
        bt2 = wpool.tile([104, 1], f32)
        nc.scalar.dma_start(bt2[0:40, :], b_d[:])
        nc.scalar.dma_start(bt2[64:104, :], b_d[:])

        chunks = [None] * NCHUNK

        def load_chunk(c):
            r0 = c * CH
            rows = min(CH, HALO - r0)
            n = rows * W
            xb = bfpool.tile([128, CH * W], f8e3, tag="xb")
            if c == 0:
                h1 = (CH // 2) * W
                nc.sync.dma_start(xb[:, 0:h1], x_d[:, 0:h1])
                nc.sync.dma_start(xb[:, h1:n], x_d[:, h1:n])
            else:
                nc.sync.dma_start(xb[:, 0:n], x_d[:, r0 * W : r0 * W + n])
            chunks[c] = xb

        # per-tap column windows: out[:, so0:so1] += W_kx^T @ in[:, si0:si1]
        CUTS = {0: (0, 511, 1, 512), 1: (0, 512, 0, 512), 2: (1, 512, 0, 511)}

        def row_slice(j, si0, si1):
            t = chunks[j // CH]
            o = (j % CH) * W
            return t[:, o + si0 : o + si1]

        loaded = 0

        # S buffers: one per 4-pair group, [128, 4W]; pair i at col i*W,
        # even pair -> partitions 0:40, odd pair -> partitions 64:104
        sbufs = {}

        def emit_dual_head(xr0, p0, xr1, p1):
            g4 = p0 // 4
            if g4 not in sbufs:
                sbufs[g4] = opool.tile([128, 4 * W], f32, tag="S", name="S")
            S = sbufs[g4]
            i0, i1 = p0 % 4, p1 % 4
            hP = hpool.tile([128, W], f32, tag="hp")
            nc.tensor.matmul(hP[0:40, :], whd[:], xr0[:], start=True, stop=True)
            nc.tensor.matmul(hP[64:104, :], whd[:], xr1[:], start=True, stop=True)
            c0 = slice(i0 * W, (i0 + 1) * W)
            c1 = slice(i1 * W, (i1 + 1) * W)
            # identity+bias on DVE, sigmoid+bias on ACT: balances the two
            # engines and halves the serialized activation chain at the tail
            nc.vector.tensor_scalar_add(S[0:32, c0], hP[0:32, :], bt2[0:32, :])
            nc.scalar.activation(S[32:40, c0], hP[32:40, :], Sigmoid, bias=bt2[32:40, :])
            nc.vector.tensor_scalar_add(S[64:96, c1], hP[64:96, :], bt2[64:96, :])
            nc.scalar.activation(S[96:104, c1], hP[96:104, :], Sigmoid, bias=bt2[96:104, :])
            if g4 == HS // 8 - 1:
                # last S group: flush per dual-head so the final tail only
                # waits on the very last pair's DMAs
                for p_, base, col in ((p0, 0, c0), (p1, 64, c1)):
                    ov1 = out_d[:, 2 * p_ * W : (2 * p_ + 2) * W].rearrange(
                        "q (e w) -> q e w", w=W
                    )
                    nc.gpsimd.dma_start(ov1[0:16, 0, :], S[base : base + 16, col])
                    nc.sync.dma_start(ov1[0:16, 1, :], S[base + 16 : base + 32, col])
                    nc.gpsimd.dma_start(ov1[16:20, 0, :], S[base + 32 : base + 36, col])
                    nc.scalar.dma_start(ov1[16:20, 1, :], S[base + 36 : base + 40, col])
                if i1 == 3:
                    sbufs.pop(g4)
            elif i1 == 3:
                flush_s(g4)

        def flush_s(g4):
            S = sbufs.pop(g4)
            y0 = 8 * g4
            # out rows y0..y0+8: [i2=2 blocks of 4 rows][pp=2 pair parity][e=2 row]
            ov = out_d[:, y0 * W : (y0 + 8) * W].rearrange(
                "q (i2 pp e w) -> q i2 pp e w", i2=2, pp=2, e=2
            )  # [20, 2, 2, 2, 512]
            Sv = S[:].rearrange("c (i2 pp w) -> c i2 pp w", i2=2, pp=2)  # [128,2,2,512]
            # spread descriptor-issue cost across three DMA-capable queues
            nc.gpsimd.dma_start(ov[0:16, :, 0, 0, :], Sv[0:16, :, 0, :])
            nc.sync.dma_start(ov[0:16, :, 0, 1, :], Sv[16:32, :, 0, :])
            nc.gpsimd.dma_start(ov[16:20, :, 0, 0, :], Sv[32:36, :, 0, :])
            nc.scalar.dma_start(ov[16:20, :, 0, 1, :], Sv[36:40, :, 0, :])
            nc.gpsimd.dma_start(ov[0:16, :, 1, 0, :], Sv[64:80, :, 1, :])
            nc.sync.dma_start(ov[0:16, :, 1, 1, :], Sv[80:96, :, 1, :])
            nc.gpsimd.dma_start(ov[16:20, :, 1, 0, :], Sv[96:100, :, 1, :])
            nc.scalar.dma_start(ov[16:20, :, 1, 1, :], Sv[100:104, :, 1, :])

        def conv_fulls(p, P):
            # each logical full matmul = two concurrent col-tiled M=64 twins
            # sharing one moving stream, so every slot in the kernel runs in
            # (128,64) tile mode and the array never pays a mode-switch drain
            b, c = 2 * p + 1, 2 * p + 2
            first = True
            for kx in (1, 0, 2):
                si0, si1, so0, so1 = CUTS[kx]
                for t_idx, j in ((0, b), (1, c)):
                    blk = 2 * kx + t_idx
                    w = wf[:, blk * 128 : (blk + 1) * 128]
                    rs = row_slice(j, si0, si1)
                    nc.tensor.matmul(
                        P[0:64, so0:so1], w[:, 0:64], rs, start=first, stop=False
                    )
                    nc.tensor.matmul(
                        P[64:128, so0:so1], w[:, 64:128], rs, start=first, stop=False
                    )
                    first = False

        def conv_halves(p, P):
            a, d = 2 * p, 2 * p + 3
            for kx in (1, 0, 2):
                si0, si1, so0, so1 = CUTS[kx]
                last = kx == 2
                nc.tensor.matmul(
                    P[0:64, so0:so1],
                    wh[:, (2 * kx) * 64 : (2 * kx + 1) * 64],
                    row_slice(a, si0, si1),
                    start=False,
                    stop=last,
                )
                nc.tensor.matmul(
                    P[64:128, so0:so1],
                    wh[:, (2 * kx + 1) * 64 : (2 * kx + 2) * 64],
                    row_slice(d, si0, si1),
                    start=False,
                    stop=last,
                )

        pend = []  # [(xr, pair_idx), ...] heads not yet emitted
        for q in range(HS // 4):  # 2-pair groups
            p0, p1 = 2 * q, 2 * q + 1
            cneed = (2 * p1 + 3) // CH
            while loaded <= min(cneed + 3, NCHUNK - 1):
                load_chunk(loaded)
                loaded += 1

            P0 = ppool.tile([128, W], f32, tag="pp")
            P1 = ppool.tile([128, W], f32, tag="pp")
            conv_fulls(p0, P0)
            conv_fulls(p1, P1)
            conv_halves(p0, P0)
            xr0 = xrpool.tile([128, W], bf16, tag="xr")
            nc.vector.tensor_scalar_max(xr0[:], P0[:], 0.0)
            conv_halves(p1, P1)
            xr1 = xrpool.tile([128, W], bf16, tag="xr")
            nc.vector.tensor_scalar_max(xr1[:], P1[:], 0.0)

            if pend:
                emit_dual_head(pend[0][0], pend[0][1], pend[1][0], pend[1][1])
            pend = [(xr0, p0), (xr1, p1)]

        emit_dual_head(pend[0][0], pend[0][1], pend[1][0], pend[1][1])

    nc.compile()
    return nc


def _get_nc():
    if "nc" not in _NC_CACHE:
        _NC_CACHE["nc"] = _build_nc()
    return _NC_CACHE["nc"]


def _pack_weights(w_shared, w_cls, b_cls, w_box, b_box, w_dir, b_dir, w_scr, b_scr):
    # fold the fp8 x pre-scale into the conv weights
    Wt = (np.ascontiguousarray(w_shared, np.float32) / XSCALE).transpose(1, 0, 2, 3)
    wfull = np.zeros((128, 6, 128), np.float32)
    whalf = np.zeros((128, 6, 64), np.float32)
    for kx in range(3):
        wfull[:, 2 * kx + 0, 0:64] = Wt[:, :, 1, kx]
        wfull[:, 2 * kx + 0, 64:128] = Wt[:, :, 0, kx]
        wfull[:, 2 * kx + 1, 0:64] = Wt[:, :, 2, kx]
        wfull[:, 2 * kx + 1, 64:128] = Wt[:, :, 1, kx]
        whalf[:, 2 * kx + 0] = Wt[:, :, 0, kx]
        whalf[:, 2 * kx + 1] = Wt[:, :, 2, kx]
    import ml_dtypes

    wfull = np.ascontiguousarray(wfull.reshape(128, 768)).astype(ml_dtypes.bfloat16)
    whalf = np.ascontiguousarray(whalf.reshape(128, 384)).astype(ml_dtypes.bfloat16)

    Wh = np.concatenate([w_cls, w_box, w_dir, w_scr], 0)[:, :, 0, 0].astype(np.float32)  # [20,64]
    bh = np.concatenate([b_cls, b_box, b_dir, b_scr], 0).astype(np.float32)  # [20]
    # head-output partition layout (sigmoid rows 32-aligned for ACT):
    #   0:16  row-y   cls/box/dir        (k 0:64)
    #   16:32 row-y+1 cls/box/dir        (k 64:128)
    #   32:36 row-y   scr                (k 0:64)
    #   36:40 row-y+1 scr                (k 64:128)
    wheads = np.zeros((128, 40), np.float32)
    wheads[0:64, 0:16] = Wh[0:16].T
    wheads[64:128, 16:32] = Wh[0:16].T
    wheads[0:64, 32:36] = Wh[16:20].T
    wheads[64:128, 36:40] = Wh[16:20].T
    wheads = wheads.astype(ml_dtypes.bfloat16)
    b40 = np.ascontiguousarray(
        np.concatenate([bh[0:16], bh[0:16], bh[16:20], bh[16:20]])[:, None]
    )  # [40,1]
    return wfull, whalf, wheads, b40


def _make_in_maps(inputs):
    import ml_dtypes

    feature = np.ascontiguousarray(inputs["feature"], np.float32)  # [4,128,512,512]
    B, Cin, H, Wd = feature.shape
    assert (B, Cin, H, Wd) == (4, 128, 512, 512)

    wfull, whalf, wheads, b40 = _pack_weights(
        np.asarray(inputs["w_shared"]),
        np.asarray(inputs["w_cls"]), np.asarray(inputs["b_cls"]),
        np.asarray(inputs["w_box"]), np.asarray(inputs["b_box"]),
        np.asarray(inputs["w_dir"]), np.asarray(inputs["b_dir"]),
        np.asarray(inputs["w_scr"]), np.asarray(inputs["b_scr"]),
    )

    f8 = np.clip(feature * XSCALE, -15.5, 15.5).astype(ml_dtypes.float8_e3m4)

    in_maps = []
    for core in range(8):
        bi, half = core // 2, core % 2
        r0 = half * HS
        xs = np.zeros((128, HALO, W), ml_dtypes.float8_e3m4)
        lo, hi = r0 - 1, r0 + HS + 1
        slo, shi = max(lo, 0), min(hi, H)
        xs[:, slo - lo : HALO - (hi - shi), :] = f8[bi, :, slo:shi, :]
        in_maps.append(
            {
                "x": xs.reshape(128, HALO * W),
                "wfull1": np.ascontiguousarray(wfull[:, 0:256]),
                "wfull2": np.ascontiguousarray(wfull[:, 256:768]),
                "whalf": whalf,
                "wheads": wheads,
                "b40": b40,
            }
        )
    return in_maps


def _gather(res):
    out = np.empty((4, 20, 512, 512), np.float32)
    for core in range(8):
        bi, half = core // 2, core % 2
        out[bi, :, half * HS : (half + 1) * HS, :] = res.results[core]["out"].reshape(
            20, HS, W
        )
    return out


def kernel(**inputs):
    from concourse.bass_utils import run_bass_kernel_spmd

    in_maps = _make_in_maps(inputs)
    nc = _get_nc()
    res = run_bass_kernel_spmd(nc, in_maps, core_ids=list(range(8)))
    return _gather(res)


def run_traced(**inputs):
    """Like kernel(), but returns (out, BassKernelResults) with a profile trace."""
    from concourse.bass_utils import run_bass_kernel_spmd

    in_maps = _make_in_maps(inputs)
    nc = _get_nc()
    res = run_bass_kernel_spmd(nc, in_maps, core_ids=list(range(8)), trace=True)
    return _gather(res), res



# revision 2
# speedup vs baseline: 1.2274x; 1.2274x over previous
"""Trainium2 Bass kernel for DetCenterDense: shared 3x3 conv + ReLU + four
1x1 head convs (cls/box/dir/scr, sigmoid on scr), channel-concatenated output.

Full inputs in / full output out. Sharding: 8 cores = batch(4) x H-halves(2);
each core computes a [20, 256, 512] output shard from a [128, 258, 512]
haloed input shard. No inter-core communication.

Compute structure (per core): the 3x3 conv is 9 shifted 1x1 convs accumulated
in PSUM, processing output rows in pairs packed into one PSUM tile [128, 512]
(partitions 0:64 = row y, 64:128 = row y+1). Every matmul in the kernel is an
M=64 column-tile so the PE array stays in (128,64) mode throughout (no
mode-switch drains) and independent M=64 matmuls run two-per-slot:

  - "full" taps (middle input rows, packed [W_a|W_b] weights) issue as two
    concurrent col-tiled twins sharing one moving stream;
  - "edge" taps (rows a/d, M=64 each) pair up in one slot;
  - two consecutive pairs' head matmuls (M=40) share one slot.

That is 9.5 tensor-engine slots per row pair -- the matmul-count floor for
K=128/M<=128 -- and the measured steady state is ~216 ns/slot (N=512 bf16
issue-gap at 2.4 GHz) with <0.2% stall.

Activations stream as fp8e3 (E3M4; x pre-scaled by 2 with 1/2 folded into the
bf16 conv weights) to halve SBUF read traffic -- without this, concurrent
DVE/ACT/DMA traffic degrades the matmul issue rate ~20%. Conv weights stay
bf16 (fp8 weights would breach the 2e-2 accuracy gate; rel l2 is ~1.35e-2).
ReLU runs on DVE; head bias on DVE (identity rows) and ACT (sigmoid rows).

Prologue/epilogue engineering (the stream itself is at the slot floor):
  - weight blocks are packed in first-use order (kx=1 taps first) so the
    small leading DMA unblocks the first LDWEIGHTS;
  - a burst of dependency-free N=64 dummy matmuls at program start keeps the
    PE busy through the input-DMA wait, releasing the HAM clock gate
    (K=4/8 -> 8/8) before the first real matmul;
  - head outputs are written channel-row-interleaved (partition 2q+e = head
    channel q of row-parity e, scr at 32+2s+e) so each row pair's whole
    [20ch x 2rows x 512] output shard flushes with ONE DMA descriptor,
    issued immediately per pair on a rotating queue -- no end-of-kernel
    descriptor backlog.
"""

import numpy as np

HS = 256          # output rows per core shard
HALO = HS + 2     # input rows per core shard (1-row halo each side)
W = 512
CH = 8            # input rows per DMA chunk
NCHUNK = (HALO + CH - 1) // CH
XSCALE = 2.0      # fp8e3 pre-scale of x; 1/XSCALE folded into w_shared
WARM_MM = 48      # dummy matmuls to release the HAM clock gate at startup

# weight-block order = first-use order (kx tap order is 1, 0, 2)
FPOS = {(1, 0): 0, (1, 1): 1, (0, 0): 2, (0, 1): 3, (2, 0): 4, (2, 1): 5}
HPOS = {1: 0, 0: 1, 2: 2}

_NC_CACHE = {}


def _build_nc():
    from contextlib import ExitStack

    import concourse.mybir as mybir
    import concourse.tile as tile
    from concourse import bacc

    f32 = mybir.dt.float32
    bf16 = mybir.dt.bfloat16
    f8e3 = mybir.dt.float8e3
    Sigmoid = mybir.ActivationFunctionType.Sigmoid

    nc = bacc.Bacc("TRN2", target_bir_lowering=False, debug=False, num_devices=8)
    x_d = nc.dram_tensor("x", [128, HALO * W], f8e3, kind="ExternalInput").ap()
    wf1_d = nc.dram_tensor("wfull1", [128, 256], bf16, kind="ExternalInput").ap()
    wf2_d = nc.dram_tensor("wfull2", [128, 512], bf16, kind="ExternalInput").ap()
    wh_d = nc.dram_tensor("whalf", [128, 6 * 64], bf16, kind="ExternalInput").ap()
    whd_d = nc.dram_tensor("wheads", [128, 40], bf16, kind="ExternalInput").ap()
    b_d = nc.dram_tensor("b40", [40, 1], f32, kind="ExternalInput").ap()
    out_d = nc.dram_tensor("out", [20, HS * W], f32, kind="ExternalOutput").ap()

    with ExitStack() as ctx:
        tc = ctx.enter_context(tile.TileContext(nc))
        wpool = ctx.enter_context(tc.tile_pool(name="w", bufs=1))
        bfpool = ctx.enter_context(tc.tile_pool(name="xbf", bufs=6))
        xrpool = ctx.enter_context(tc.tile_pool(name="xr", bufs=4))
        opool = ctx.enter_context(tc.tile_pool(name="ot", bufs=3))
        ppool = ctx.enter_context(tc.tile_pool(name="pp", bufs=5, space="PSUM"))
        hpool = ctx.enter_context(tc.tile_pool(name="hp", bufs=2, space="PSUM"))
        wupool = ctx.enter_context(tc.tile_pool(name="wu", bufs=1, space="PSUM"))

        # HAM warmup: dependency-free dummy matmuls issued while the weight/x
        # DMAs are in flight, so the PE clock gate opens before real work
        dum = wpool.tile([128, 64], bf16)
        nc.gpsimd.memset(dum[:], 0.0)
        dP = wupool.tile([64, 64], f32)
        for _ in range(WARM_MM):
            nc.tensor.matmul(dP[:], dum[:], dum[:], start=True, stop=True)

        # weights go on the scalar queue so their descriptor-issue cost does
        # not delay the x-chunk DMAs on the sync queue (first matmul latency);
        # wf1_d carries the kx=1 blocks (first matmuls' weights) so their
        # small DMA completes early and the first LDWEIGHTS releases
        wf = wpool.tile([128, 6 * 128], bf16)
        nc.scalar.dma_start(wf[:, 0:256], wf1_d[:])
        nc.scalar.dma_start(wf[:, 256:768], wf2_d[:])
        wh = wpool.tile([128, 6 * 64], bf16)
        nc.scalar.dma_start(wh[:], wh_d[:])
        whd = wpool.tile([128, 40], bf16)
        nc.scalar.dma_start(whd[:], whd_d[:])
        bt2 = wpool.tile([104, 1], f32)
        nc.scalar.dma_start(bt2[0:40, :], b_d[:])
        nc.scalar.dma_start(bt2[64:104, :], b_d[:])

        chunks = [None] * NCHUNK

        def load_chunk(c):
            r0 = c * CH
            rows = min(CH, HALO - r0)
            n = rows * W
            xb = bfpool.tile([128, CH * W], f8e3, tag="xb")
            if c == 0:
                # rows 0:3 cover the very first matmuls (pair 0 reads rows
                # 0..3); a small leading DMA lands them earlier
                h1 = 3 * W
                nc.sync.dma_start(xb[:, 0:h1], x_d[:, 0:h1])
                nc.sync.dma_start(xb[:, h1:n], x_d[:, h1:n])
            else:
                nc.sync.dma_start(xb[:, 0:n], x_d[:, r0 * W : r0 * W + n])
            chunks[c] = xb

        # per-tap column windows: out[:, so0:so1] += W_kx^T @ in[:, si0:si1]
        CUTS = {0: (0, 511, 1, 512), 1: (0, 512, 0, 512), 2: (1, 512, 0, 511)}

        def row_slice(j, si0, si1):
            t = chunks[j // CH]
            o = (j % CH) * W
            return t[:, o + si0 : o + si1]

        loaded = 0

        flushq = (nc.gpsimd, nc.sync, nc.scalar)

        def emit_dual_head(xr0, p0, xr1, p1):
            # head-output partition layout (channel-row-interleaved):
            #   base+2q+e   = head channel q (cls/box/dir), row-parity e
            #   base+32+2s+e = scr channel s (sigmoid; base+32 is 32-aligned
            #   for ACT), so S[base:base+40] maps 1:1 onto the pair's
            #   [20, 2, 512] output block -> single-descriptor flush
            S = opool.tile([128, 2 * W], f32, tag="S", name="S")
            hP = hpool.tile([128, W], f32, tag="hp")
            nc.tensor.matmul(hP[0:40, :], whd[:], xr0[:], start=True, stop=True)
            nc.tensor.matmul(hP[64:104, :], whd[:], xr1[:], start=True, stop=True)
            for base, col, p_ in ((0, 0, p0), (64, W, p1)):
                cs = slice(col, col + W)
                nc.vector.tensor_scalar_add(
                    S[base : base + 32, cs], hP[base : base + 32, :], bt2[base : base + 32, :]
                )
                nc.scalar.activation(
                    S[base + 32 : base + 40, cs],
                    hP[base + 32 : base + 40, :],
                    Sigmoid,
                    bias=bt2[base + 32 : base + 40, :],
                )
                flushq[p_ % 3].dma_start(
                    out_d[:, 2 * p_ * W : (2 * p_ + 2) * W], S[base : base + 40, cs]
                )

        def conv_fulls(p, P):
            # each logical full matmul = two concurrent col-tiled M=64 twins
            # sharing one moving stream, so every slot in the kernel runs in
            # (128,64) tile mode and the array never pays a mode-switch drain
            b, c = 2 * p + 1, 2 * p + 2
            first = True
            for kx in (1, 0, 2):
                si0, si1, so0, so1 = CUTS[kx]
                for t_idx, j in ((0, b), (1, c)):
                    blk = FPOS[(kx, t_idx)]
                    w = wf[:, blk * 128 : (blk + 1) * 128]
                    rs = row_slice(j, si0, si1)
                    nc.tensor.matmul(
                        P[0:64, so0:so1], w[:, 0:64], rs, start=first, stop=False
                    )
                    nc.tensor.matmul(
                        P[64:128, so0:so1], w[:, 64:128], rs, start=first, stop=False
                    )
                    first = False

        def conv_halves(p, P):
            a, d = 2 * p, 2 * p + 3
            for kx in (1, 0, 2):
                si0, si1, so0, so1 = CUTS[kx]
                last = kx == 2
                hb = 2 * HPOS[kx]
                nc.tensor.matmul(
                    P[0:64, so0:so1],
                    wh[:, hb * 64 : (hb + 1) * 64],
                    row_slice(a, si0, si1),
                    start=False,
                    stop=last,
                )
                nc.tensor.matmul(
                    P[64:128, so0:so1],
                    wh[:, (hb + 1) * 64 : (hb + 2) * 64],
                    row_slice(d, si0, si1),
                    start=False,
                    stop=last,
                )

        pend = []  # [(xr, pair_idx), ...] heads not yet emitted
        for q in range(HS // 4):  # 2-pair groups
            p0, p1 = 2 * q, 2 * q + 1
            cneed = (2 * p1 + 3) // CH
            while loaded <= min(cneed + 3, NCHUNK - 1):
                load_chunk(loaded)
                loaded += 1

            P0 = ppool.tile([128, W], f32, tag="pp")
            P1 = ppool.tile([128, W], f32, tag="pp")
            conv_fulls(p0, P0)
            conv_fulls(p1, P1)
            conv_halves(p0, P0)
            xr0 = xrpool.tile([128, W], bf16, tag="xr")
            nc.vector.tensor_scalar_max(xr0[:], P0[:], 0.0)
            conv_halves(p1, P1)
            xr1 = xrpool.tile([128, W], bf16, tag="xr")
            nc.vector.tensor_scalar_max(xr1[:], P1[:], 0.0)

            if pend:
                emit_dual_head(pend[0][0], pend[0][1], pend[1][0], pend[1][1])
            pend = [(xr0, p0), (xr1, p1)]

        emit_dual_head(pend[0][0], pend[0][1], pend[1][0], pend[1][1])

    nc.compile()
    return nc


def _get_nc():
    if "nc" not in _NC_CACHE:
        _NC_CACHE["nc"] = _build_nc()
    return _NC_CACHE["nc"]


def _pack_weights(w_shared, w_cls, b_cls, w_box, b_box, w_dir, b_dir, w_scr, b_scr):
    # fold the fp8 x pre-scale into the conv weights
    Wt = (np.ascontiguousarray(w_shared, np.float32) / XSCALE).transpose(1, 0, 2, 3)
    wfull = np.zeros((128, 6, 128), np.float32)
    whalf = np.zeros((128, 6, 64), np.float32)
    for kx in range(3):
        wfull[:, FPOS[(kx, 0)], 0:64] = Wt[:, :, 1, kx]
        wfull[:, FPOS[(kx, 0)], 64:128] = Wt[:, :, 0, kx]
        wfull[:, FPOS[(kx, 1)], 0:64] = Wt[:, :, 2, kx]
        wfull[:, FPOS[(kx, 1)], 64:128] = Wt[:, :, 1, kx]
        whalf[:, 2 * HPOS[kx] + 0] = Wt[:, :, 0, kx]
        whalf[:, 2 * HPOS[kx] + 1] = Wt[:, :, 2, kx]
    import ml_dtypes

    wfull = np.ascontiguousarray(wfull.reshape(128, 768)).astype(ml_dtypes.bfloat16)
    whalf = np.ascontiguousarray(whalf.reshape(128, 384)).astype(ml_dtypes.bfloat16)

    Wh = np.concatenate([w_cls, w_box, w_dir, w_scr], 0)[:, :, 0, 0].astype(np.float32)  # [20,64]
    bh = np.concatenate([b_cls, b_box, b_dir, b_scr], 0).astype(np.float32)  # [20]
    # channel-row-interleaved head layout: partition 2q+e = channel q of
    # row-parity e (k-half e holds that row's 64 conv channels); scr channels
    # at 32+2s+e so the sigmoid rows start 32-aligned for ACT
    wheads = np.zeros((128, 40), np.float32)
    for qq in range(16):
        wheads[0:64, 2 * qq] = Wh[qq]
        wheads[64:128, 2 * qq + 1] = Wh[qq]
    for s in range(4):
        wheads[0:64, 32 + 2 * s] = Wh[16 + s]
        wheads[64:128, 33 + 2 * s] = Wh[16 + s]
    wheads = wheads.astype(ml_dtypes.bfloat16)
    b40 = np.empty((40,), np.float32)
    b40[0:32] = np.repeat(bh[0:16], 2)
    b40[32:40] = np.repeat(bh[16:20], 2)
    b40 = np.ascontiguousarray(b40[:, None])  # [40,1]
    return wfull, whalf, wheads, b40


def _make_in_maps(inputs):
    import ml_dtypes

    feature = np.ascontiguousarray(inputs["feature"], np.float32)  # [4,128,512,512]
    B, Cin, H, Wd = feature.shape
    assert (B, Cin, H, Wd) == (4, 128, 512, 512)

    wfull, whalf, wheads, b40 = _pack_weights(
        np.asarray(inputs["w_shared"]),
        np.asarray(inputs["w_cls"]), np.asarray(inputs["b_cls"]),
        np.asarray(inputs["w_box"]), np.asarray(inputs["b_box"]),
        np.asarray(inputs["w_dir"]), np.asarray(inputs["b_dir"]),
        np.asarray(inputs["w_scr"]), np.asarray(inputs["b_scr"]),
    )

    f8 = np.clip(feature * XSCALE, -15.5, 15.5).astype(ml_dtypes.float8_e3m4)

    in_maps = []
    for core in range(8):
        bi, half = core // 2, core % 2
        r0 = half * HS
        xs = np.zeros((128, HALO, W), ml_dtypes.float8_e3m4)
        lo, hi = r0 - 1, r0 + HS + 1
        slo, shi = max(lo, 0), min(hi, H)
        xs[:, slo - lo : HALO - (hi - shi), :] = f8[bi, :, slo:shi, :]
        in_maps.append(
            {
                "x": xs.reshape(128, HALO * W),
                "wfull1": np.ascontiguousarray(wfull[:, 0:256]),
                "wfull2": np.ascontiguousarray(wfull[:, 256:768]),
                "whalf": whalf,
                "wheads": wheads,
                "b40": b40,
            }
        )
    return in_maps


def _gather(res):
    out = np.empty((4, 20, 512, 512), np.float32)
    for core in range(8):
        bi, half = core // 2, core % 2
        out[bi, :, half * HS : (half + 1) * HS, :] = res.results[core]["out"].reshape(
            20, HS, W
        )
    return out


def kernel(**inputs):
    from concourse.bass_utils import run_bass_kernel_spmd

    in_maps = _make_in_maps(inputs)
    nc = _get_nc()
    res = run_bass_kernel_spmd(nc, in_maps, core_ids=list(range(8)))
    return _gather(res)


def run_traced(**inputs):
    """Like kernel(), but returns (out, BassKernelResults) with a profile trace."""
    from concourse.bass_utils import run_bass_kernel_spmd

    in_maps = _make_in_maps(inputs)
    nc = _get_nc()
    res = run_bass_kernel_spmd(nc, in_maps, core_ids=list(range(8)), trace=True)
    return _gather(res), res


# revision 11
# speedup vs baseline: 1.2302x; 1.0023x over previous
"""Trainium2 Bass kernel for DetCenterDense: shared 3x3 conv + ReLU + four
1x1 head convs (cls/box/dir/scr, sigmoid on scr), channel-concatenated output.

Full inputs in / full output out. Sharding: 8 cores = batch(4) x H-halves(2);
each core computes a [20, 256, 512] output shard from a [128, 258, 512]
haloed input shard. No inter-core communication.

Compute structure (per core): the 3x3 conv is 9 shifted 1x1 convs accumulated
in PSUM, processing output rows in pairs packed into one PSUM tile [128, 512]
(partitions 0:64 = row y, 64:128 = row y+1). Every matmul in the kernel is an
M=64 column-tile so the PE array stays in (128,64) mode throughout (no
mode-switch drains) and independent M=64 matmuls run two-per-slot:

  - "full" taps (middle input rows, packed [W_a|W_b] weights) issue as two
    concurrent col-tiled twins sharing one moving stream;
  - "edge" taps (rows a/d, M=64 each) pair up in one slot;
  - two consecutive pairs' head matmuls (M=40) share one slot.

That is 9.5 tensor-engine slots per row pair -- the matmul-count floor for
K=128/M<=128 -- and the measured steady state is ~216 ns/slot (N=512 bf16
issue-gap at 2.4 GHz) with <0.2% stall.

Activations stream as fp8e3 (E3M4; x pre-scaled by 2 with 1/2 folded into the
bf16 conv weights) to halve SBUF read traffic -- without this, concurrent
DVE/ACT/DMA traffic degrades the matmul issue rate ~20%. Conv weights stay
bf16 (fp8 weights would breach the 2e-2 accuracy gate; rel l2 is ~1.35e-2).
ReLU runs on DVE; head bias on DVE (identity rows) and ACT (sigmoid rows).

Prologue/epilogue engineering (the stream itself is at the slot floor):
  - weight blocks are packed in first-use order (kx=1 taps first) so the
    small leading DMA unblocks the first LDWEIGHTS;
  - a burst of dependency-free N=64 dummy matmuls at program start keeps the
    PE busy through the input-DMA wait, releasing the HAM clock gate
    (K=4/8 -> 8/8) before the first real matmul;
  - head outputs are written channel-row-interleaved (partition 2q+e = head
    channel q of row-parity e, scr at 32+2s+e) so each row pair's whole
    [20ch x 2rows x 512] output shard flushes with ONE DMA descriptor,
    issued immediately per pair on a rotating queue -- no end-of-kernel
    descriptor backlog.
"""

import numpy as np

HS = 256          # output rows per core shard
HALO = HS + 2     # input rows per core shard (1-row halo each side)
W = 512
CH = 8            # input rows per DMA chunk
NCHUNK = (HALO + CH - 1) // CH
XSCALE = 2.0      # fp8e3 pre-scale of x; 1/XSCALE folded into w_shared
WARM_MM = 56      # dummy matmuls to release the HAM clock gate at startup

# all conv/head weights ship as ONE bf16 dram tensor, column layout in
# first-use (kx-major) order so three per-kx DMA descriptors stream in just
# ahead of the matmuls that need them (kx tap order is 1, 0, 2):
#   [kx1 fulls 256][kx1 halves 128][kx0 fulls 256][kx0 halves 128]
#   [kx2 fulls 256][kx2 halves 128][heads 40]  -> 1192 cols
FULLS = {1: 0, 0: 384, 2: 768}     # col base of the 2x128 full blocks per kx
HALVES = {1: 256, 0: 640, 2: 1024} # col base of the 2x64 half blocks per kx
WHD0 = 1152
WALLC = 1192

_NC_CACHE = {}


def _build_nc():
    from contextlib import ExitStack

    import concourse.mybir as mybir
    import concourse.tile as tile
    from concourse import bacc

    f32 = mybir.dt.float32
    bf16 = mybir.dt.bfloat16
    f8e3 = mybir.dt.float8e3
    Sigmoid = mybir.ActivationFunctionType.Sigmoid

    nc = bacc.Bacc("TRN2", target_bir_lowering=False, debug=False, num_devices=8)
    x_d = nc.dram_tensor("x", [128, HALO * W], f8e3, kind="ExternalInput").ap()
    wall_d = nc.dram_tensor("wall", [128, WALLC], bf16, kind="ExternalInput").ap()
    b_d = nc.dram_tensor("b40", [40, 1], f32, kind="ExternalInput").ap()
    out_d = nc.dram_tensor("out", [20, HS * W], f32, kind="ExternalOutput").ap()

    with ExitStack() as ctx:
        tc = ctx.enter_context(tile.TileContext(nc))
        wpool = ctx.enter_context(tc.tile_pool(name="w", bufs=1))
        bfpool = ctx.enter_context(tc.tile_pool(name="xbf", bufs=6))
        xrpool = ctx.enter_context(tc.tile_pool(name="xr", bufs=4))
        opool = ctx.enter_context(tc.tile_pool(name="ot", bufs=3))
        ppool = ctx.enter_context(tc.tile_pool(name="pp", bufs=5, space="PSUM"))
        hpool = ctx.enter_context(tc.tile_pool(name="hp", bufs=2, space="PSUM"))
        wupool = ctx.enter_context(tc.tile_pool(name="wu", bufs=1, space="PSUM"))

        # HAM warmup: dependency-free dummy matmuls issued while the weight/x
        # DMAs are in flight, so the PE clock gate opens before real work
        dum = wpool.tile([128, 64], bf16)
        nc.gpsimd.memset(dum[:], 0.0)
        dP = wupool.tile([64, 64], f32)
        for _ in range(WARM_MM):
            nc.tensor.matmul(dP[:], dum[:], dum[:], start=True, stop=True)

        # weights go on the scalar queue so their descriptor-issue cost does
        # not delay the x-chunk DMAs on the sync queue (first matmul latency);
        # one descriptor per kx group, in tap order, so each lands just
        # before its matmuls need it
        wall = wpool.tile([128, WALLC], bf16)
        nc.scalar.dma_start(wall[:, 0:384], wall_d[:, 0:384])
        nc.scalar.dma_start(wall[:, 384:768], wall_d[:, 384:768])
        nc.scalar.dma_start(wall[:, 768:WALLC], wall_d[:, 768:WALLC])
        whd = wall[:, WHD0 : WHD0 + 40]
        bt2 = wpool.tile([104, 1], f32)
        nc.scalar.dma_start(bt2[0:40, :], b_d[:])
        nc.scalar.dma_start(bt2[64:104, :], b_d[:])

        chunks = [None] * NCHUNK

        def load_chunk(c):
            r0 = c * CH
            rows = min(CH, HALO - r0)
            n = rows * W
            xb = bfpool.tile([128, CH * W], f8e3, tag="xb")
            if c == 0:
                # rows 0:4 cover the first kx=1 round of group 0; a smaller
                # leading DMA lands them earlier
                h1 = 4 * W
                nc.sync.dma_start(xb[:, 0:h1], x_d[:, 0:h1])
                nc.sync.dma_start(xb[:, h1:n], x_d[:, h1:n])
            else:
                nc.sync.dma_start(xb[:, 0:n], x_d[:, r0 * W : r0 * W + n])
            chunks[c] = xb

        # per-tap column windows: out[:, so0:so1] += W_kx^T @ in[:, si0:si1]
        CUTS = {0: (0, 511, 1, 512), 1: (0, 512, 0, 512), 2: (1, 512, 0, 511)}

        def row_slice(j, si0, si1):
            t = chunks[j // CH]
            o = (j % CH) * W
            return t[:, o + si0 : o + si1]

        loaded = 0

        # gpsimd sits out the last 4 pairs so its (expensive) SWDGE teardown
        # drain overlaps the matmul stream instead of trailing the kernel
        def flushq(p_):
            if p_ >= HS // 2 - 4:
                return (nc.sync, nc.scalar)[p_ % 2]
            return (nc.gpsimd, nc.sync, nc.scalar)[p_ % 3]

        def emit_dual_head(xr0, p0, xr1, p1):
            # head-output partition layout (channel-row-interleaved):
            #   base+2q+e   = head channel q (cls/box/dir), row-parity e
            #   base+32+2s+e = scr channel s (sigmoid; base+32 is 32-aligned
            #   for ACT), so S[base:base+40] maps 1:1 onto the pair's
            #   [20, 2, 512] output block -> single-descriptor flush
            S = opool.tile([128, 2 * W], f32, tag="S", name="S")
            hP = hpool.tile([128, W], f32, tag="hp")
            nc.tensor.matmul(hP[0:40, :], whd[:], xr0[:], start=True, stop=True)
            nc.tensor.matmul(hP[64:104, :], whd[:], xr1[:], start=True, stop=True)
            for base, col, p_ in ((0, 0, p0), (64, W, p1)):
                cs = slice(col, col + W)
                nc.vector.tensor_scalar_add(
                    S[base : base + 32, cs], hP[base : base + 32, :], bt2[base : base + 32, :]
                )
                nc.scalar.activation(
                    S[base + 32 : base + 40, cs],
                    hP[base + 32 : base + 40, :],
                    Sigmoid,
                    bias=bt2[base + 32 : base + 40, :],
                )
                flushq(p_).dma_start(
                    out_d[:, 2 * p_ * W : (2 * p_ + 2) * W], S[base : base + 40, cs]
                )

        def conv_fulls_kx(p, P, kx, first):
            # each logical full matmul = two concurrent col-tiled M=64 twins
            # sharing one moving stream, so every slot in the kernel runs in
            # (128,64) tile mode and the array never pays a mode-switch drain
            b, c = 2 * p + 1, 2 * p + 2
            si0, si1, so0, so1 = CUTS[kx]
            for t_idx, j in ((0, b), (1, c)):
                w = wall[:, FULLS[kx] + t_idx * 128 : FULLS[kx] + (t_idx + 1) * 128]
                rs = row_slice(j, si0, si1)
                nc.tensor.matmul(
                    P[0:64, so0:so1], w[:, 0:64], rs, start=first, stop=False
                )
                nc.tensor.matmul(
                    P[64:128, so0:so1], w[:, 64:128], rs, start=first, stop=False
                )
                first = False

        def conv_halves_kx(p, P, kx):
            a, d = 2 * p, 2 * p + 3
            si0, si1, so0, so1 = CUTS[kx]
            last = kx == 2
            hb = HALVES[kx]
            nc.tensor.matmul(
                P[0:64, so0:so1],
                wall[:, hb : hb + 64],
                row_slice(a, si0, si1),
                start=False,
                stop=last,
            )
            nc.tensor.matmul(
                P[64:128, so0:so1],
                wall[:, hb + 64 : hb + 128],
                row_slice(d, si0, si1),
                start=False,
                stop=last,
            )

        pend = []  # [(xr, pair_idx), ...] heads not yet emitted
        for q in range(HS // 4):  # 2-pair groups
            p0, p1 = 2 * q, 2 * q + 1
            cneed = (2 * p1 + 3) // CH
            while loaded <= min(cneed + 3, NCHUNK - 1):
                load_chunk(loaded)
                loaded += 1

            P0 = ppool.tile([128, W], f32, tag="pp")
            P1 = ppool.tile([128, W], f32, tag="pp")
            if q == 0:
                # kx-major order for the first group: 6 slots of kx=1-only
                # work cover the in-flight kx=0/kx=2 weight descriptors
                for kx in (1, 0, 2):
                    conv_fulls_kx(p0, P0, kx, first=kx == 1)
                    conv_halves_kx(p0, P0, kx)
                    conv_fulls_kx(p1, P1, kx, first=kx == 1)
                    conv_halves_kx(p1, P1, kx)
                xr0 = xrpool.tile([128, W], bf16, tag="xr")
                nc.vector.tensor_scalar_max(xr0[:], P0[:], 0.0)
                xr1 = xrpool.tile([128, W], bf16, tag="xr")
                nc.vector.tensor_scalar_max(xr1[:], P1[:], 0.0)
            else:
                for kx in (1, 0, 2):
                    conv_fulls_kx(p0, P0, kx, first=kx == 1)
                for kx in (1, 0, 2):
                    conv_fulls_kx(p1, P1, kx, first=kx == 1)
                for kx in (1, 0, 2):
                    conv_halves_kx(p0, P0, kx)
                xr0 = xrpool.tile([128, W], bf16, tag="xr")
                nc.vector.tensor_scalar_max(xr0[:], P0[:], 0.0)
                for kx in (1, 0, 2):
                    conv_halves_kx(p1, P1, kx)
                xr1 = xrpool.tile([128, W], bf16, tag="xr")
                nc.vector.tensor_scalar_max(xr1[:], P1[:], 0.0)

            if pend:
                emit_dual_head(pend[0][0], pend[0][1], pend[1][0], pend[1][1])
            pend = [(xr0, p0), (xr1, p1)]

        emit_dual_head(pend[0][0], pend[0][1], pend[1][0], pend[1][1])

    nc.compile()
    return nc


def _get_nc():
    if "nc" not in _NC_CACHE:
        _NC_CACHE["nc"] = _build_nc()
    return _NC_CACHE["nc"]


def _pack_weights(w_shared, w_cls, b_cls, w_box, b_box, w_dir, b_dir, w_scr, b_scr):
    # fold the fp8 x pre-scale into the conv weights
    Wt = (np.ascontiguousarray(w_shared, np.float32) / XSCALE).transpose(1, 0, 2, 3)
    wall = np.zeros((128, WALLC), np.float32)
    for kx in range(3):
        f0 = FULLS[kx]
        wall[:, f0 + 0 : f0 + 64] = Wt[:, :, 1, kx]
        wall[:, f0 + 64 : f0 + 128] = Wt[:, :, 0, kx]
        wall[:, f0 + 128 : f0 + 192] = Wt[:, :, 2, kx]
        wall[:, f0 + 192 : f0 + 256] = Wt[:, :, 1, kx]
        h0 = HALVES[kx]
        wall[:, h0 + 0 : h0 + 64] = Wt[:, :, 0, kx]
        wall[:, h0 + 64 : h0 + 128] = Wt[:, :, 2, kx]

    Wh = np.concatenate([w_cls, w_box, w_dir, w_scr], 0)[:, :, 0, 0].astype(np.float32)  # [20,64]
    bh = np.concatenate([b_cls, b_box, b_dir, b_scr], 0).astype(np.float32)  # [20]
    # channel-row-interleaved head layout: partition 2q+e = channel q of
    # row-parity e (k-half e holds that row's 64 conv channels); scr channels
    # at 32+2s+e so the sigmoid rows start 32-aligned for ACT
    for qq in range(16):
        wall[0:64, WHD0 + 2 * qq] = Wh[qq]
        wall[64:128, WHD0 + 2 * qq + 1] = Wh[qq]
    for s in range(4):
        wall[0:64, WHD0 + 32 + 2 * s] = Wh[16 + s]
        wall[64:128, WHD0 + 33 + 2 * s] = Wh[16 + s]
    import ml_dtypes

    wall = np.ascontiguousarray(wall).astype(ml_dtypes.bfloat16)
    b40 = np.empty((40,), np.float32)
    b40[0:32] = np.repeat(bh[0:16], 2)
    b40[32:40] = np.repeat(bh[16:20], 2)
    b40 = np.ascontiguousarray(b40[:, None])  # [40,1]
    return wall, b40


def _make_in_maps(inputs):
    import ml_dtypes

    feature = np.ascontiguousarray(inputs["feature"], np.float32)  # [4,128,512,512]
    B, Cin, H, Wd = feature.shape
    assert (B, Cin, H, Wd) == (4, 128, 512, 512)

    wall, b40 = _pack_weights(
        np.asarray(inputs["w_shared"]),
        np.asarray(inputs["w_cls"]), np.asarray(inputs["b_cls"]),
        np.asarray(inputs["w_box"]), np.asarray(inputs["b_box"]),
        np.asarray(inputs["w_dir"]), np.asarray(inputs["b_dir"]),
        np.asarray(inputs["w_scr"]), np.asarray(inputs["b_scr"]),
    )

    f8 = np.clip(feature * XSCALE, -15.5, 15.5).astype(ml_dtypes.float8_e3m4)

    in_maps = []
    for core in range(8):
        bi, half = core // 2, core % 2
        r0 = half * HS
        xs = np.zeros((128, HALO, W), ml_dtypes.float8_e3m4)
        lo, hi = r0 - 1, r0 + HS + 1
        slo, shi = max(lo, 0), min(hi, H)
        xs[:, slo - lo : HALO - (hi - shi), :] = f8[bi, :, slo:shi, :]
        in_maps.append(
            {
                "x": xs.reshape(128, HALO * W),
                "wall": wall,
                "b40": b40,
            }
        )
    return in_maps


def _gather(res):
    out = np.empty((4, 20, 512, 512), np.float32)
    for core in range(8):
        bi, half = core // 2, core % 2
        out[bi, :, half * HS : (half + 1) * HS, :] = res.results[core]["out"].reshape(
            20, HS, W
        )
    return out


def kernel(**inputs):
    from concourse.bass_utils import run_bass_kernel_spmd

    in_maps = _make_in_maps(inputs)
    nc = _get_nc()
    res = run_bass_kernel_spmd(nc, in_maps, core_ids=list(range(8)))
    return _gather(res)


def run_traced(**inputs):
    """Like kernel(), but returns (out, BassKernelResults) with a profile trace."""
    from concourse.bass_utils import run_bass_kernel_spmd

    in_maps = _make_in_maps(inputs)
    nc = _get_nc()
    res = run_bass_kernel_spmd(nc, in_maps, core_ids=list(range(8)), trace=True)
    return _gather(res), res


# revision 15
# speedup vs baseline: 1.2314x; 1.0010x over previous
"""Trainium2 Bass kernel for DetCenterDense: shared 3x3 conv + ReLU + four
1x1 head convs (cls/box/dir/scr, sigmoid on scr), channel-concatenated output.

Full inputs in / full output out. Sharding: 8 cores = batch(4) x H-halves(2);
each core computes a [20, 256, 512] output shard from a [128, 258, 512]
haloed input shard. No inter-core communication.

Compute structure (per core): the 3x3 conv is 9 shifted 1x1 convs accumulated
in PSUM, processing output rows in pairs packed into one PSUM tile [128, 512]
(partitions 0:64 = row y, 64:128 = row y+1). Every matmul in the kernel is an
M=64 column-tile so the PE array stays in (128,64) mode throughout (no
mode-switch drains) and independent M=64 matmuls run two-per-slot:

  - "full" taps (middle input rows, packed [W_a|W_b] weights) issue as two
    concurrent col-tiled twins sharing one moving stream;
  - "edge" taps (rows a/d, M=64 each) pair up in one slot;
  - two consecutive pairs' head matmuls (M=40) share one slot.

That is 9.5 tensor-engine slots per row pair -- the matmul-count floor for
K=128/M<=128 -- and the measured steady state is ~216 ns/slot (N=512 bf16
issue-gap at 2.4 GHz) with <0.2% stall.

Activations stream as fp8e3 (E3M4; x pre-scaled by 2 with 1/2 folded into the
bf16 conv weights) to halve SBUF read traffic -- without this, concurrent
DVE/ACT/DMA traffic degrades the matmul issue rate ~20%. Conv weights stay
bf16 (fp8 weights would breach the 2e-2 accuracy gate; rel l2 is ~1.35e-2).
ReLU runs on DVE; head bias on DVE (identity rows) and ACT (sigmoid rows).

Prologue/epilogue engineering (the stream itself is at the slot floor):
  - weight blocks are packed in first-use order (kx=1 taps first) so the
    small leading DMA unblocks the first LDWEIGHTS;
  - a burst of dependency-free N=64 dummy matmuls at program start keeps the
    PE busy through the input-DMA wait, releasing the HAM clock gate
    (K=4/8 -> 8/8) before the first real matmul;
  - head outputs are written channel-row-interleaved (partition 2q+e = head
    channel q of row-parity e, scr at 32+2s+e) so each row pair's whole
    [20ch x 2rows x 512] output shard flushes with ONE DMA descriptor,
    issued immediately per pair on a rotating queue -- no end-of-kernel
    descriptor backlog.
"""

import numpy as np

HS = 256          # output rows per core shard
HALO = HS + 2     # input rows per core shard (1-row halo each side)
W = 512
CH = 8            # input rows per DMA chunk
NCHUNK = (HALO + CH - 1) // CH
XSCALE = 2.0      # fp8e3 pre-scale of x; 1/XSCALE folded into w_shared
WARM_MM = 56      # dummy matmuls to release the HAM clock gate at startup

# all conv/head weights ship as ONE bf16 dram tensor, column layout in
# first-use (kx-major) order so three per-kx DMA descriptors stream in just
# ahead of the matmuls that need them (kx tap order is 1, 0, 2):
#   [kx1 fulls 256][kx1 halves 128][kx0 fulls 256][kx0 halves 128]
#   [kx2 fulls 256][kx2 halves 128][heads 40]  -> 1192 cols
FULLS = {1: 0, 0: 384, 2: 768}     # col base of the 2x128 full blocks per kx
HALVES = {1: 256, 0: 640, 2: 1024} # col base of the 2x64 half blocks per kx
WHD0 = 1152
WALLC = 1192

_NC_CACHE = {}


def _build_nc():
    from contextlib import ExitStack

    import concourse.mybir as mybir
    import concourse.tile as tile
    from concourse import bacc

    f32 = mybir.dt.float32
    bf16 = mybir.dt.bfloat16
    f8e3 = mybir.dt.float8e3
    Sigmoid = mybir.ActivationFunctionType.Sigmoid

    nc = bacc.Bacc("TRN2", target_bir_lowering=False, debug=False, num_devices=8)
    x_d = nc.dram_tensor("x", [128, HALO * W], f8e3, kind="ExternalInput").ap()
    wall_d = nc.dram_tensor("wall", [128, WALLC], bf16, kind="ExternalInput").ap()
    b_d = nc.dram_tensor("b40", [40, 1], f32, kind="ExternalInput").ap()
    out_d = nc.dram_tensor("out", [20, HS * W], f32, kind="ExternalOutput").ap()

    with ExitStack() as ctx:
        tc = ctx.enter_context(tile.TileContext(nc))
        wpool = ctx.enter_context(tc.tile_pool(name="w", bufs=1))
        bfpool = ctx.enter_context(tc.tile_pool(name="xbf", bufs=6))
        xrpool = ctx.enter_context(tc.tile_pool(name="xr", bufs=4))
        opool = ctx.enter_context(tc.tile_pool(name="ot", bufs=3))
        ppool = ctx.enter_context(tc.tile_pool(name="pp", bufs=5, space="PSUM"))
        hpool = ctx.enter_context(tc.tile_pool(name="hp", bufs=2, space="PSUM"))
        wupool = ctx.enter_context(tc.tile_pool(name="wu", bufs=1, space="PSUM"))

        # HAM warmup: dependency-free dummy matmuls issued while the weight/x
        # DMAs are in flight, so the PE clock gate opens before real work
        dum = wpool.tile([128, 64], bf16)
        nc.gpsimd.memset(dum[:], 0.0)
        dP = wupool.tile([64, 64], f32)
        for _ in range(WARM_MM):
            nc.tensor.matmul(dP[:], dum[:], dum[:], start=True, stop=True)

        # weights go on the scalar queue so their descriptor-issue cost does
        # not delay the x-chunk DMAs on the sync queue (first matmul latency);
        # one descriptor per kx group, in tap order, so each lands just
        # before its matmuls need it
        # kx=1 weights ride the otherwise-idle gpsimd queue (in two pieces so
        # the first fulls unblock the first LDWEIGHTS asap); kx=0/kx=2 load
        # in parallel on the scalar queue
        wall = wpool.tile([128, WALLC], bf16)
        nc.gpsimd.dma_start(wall[:, 0:256], wall_d[:, 0:256])
        nc.gpsimd.dma_start(wall[:, 256:384], wall_d[:, 256:384])
        nc.scalar.dma_start(wall[:, 384:768], wall_d[:, 384:768])
        nc.scalar.dma_start(wall[:, 768:WALLC], wall_d[:, 768:WALLC])
        whd = wall[:, WHD0 : WHD0 + 40]
        bt2 = wpool.tile([104, 1], f32)
        nc.scalar.dma_start(bt2[0:40, :], b_d[:])
        nc.scalar.dma_start(bt2[64:104, :], b_d[:])

        chunks = [None] * NCHUNK

        def load_chunk(c):
            r0 = c * CH
            rows = min(CH, HALO - r0)
            n = rows * W
            xb = bfpool.tile([128, CH * W], f8e3, tag="xb")
            if c == 0:
                # group 0's kx-major order consumes rows 0..5 within six
                # slots; three ascending DMAs land them just in time
                nc.sync.dma_start(xb[:, 0 : 3 * W], x_d[:, 0 : 3 * W])
                nc.sync.dma_start(xb[:, 3 * W : 6 * W], x_d[:, 3 * W : 6 * W])
                nc.sync.dma_start(xb[:, 6 * W : n], x_d[:, 6 * W : n])
            else:
                nc.sync.dma_start(xb[:, 0:n], x_d[:, r0 * W : r0 * W + n])
            chunks[c] = xb

        # per-tap column windows: out[:, so0:so1] += W_kx^T @ in[:, si0:si1]
        CUTS = {0: (0, 511, 1, 512), 1: (0, 512, 0, 512), 2: (1, 512, 0, 511)}

        def row_slice(j, si0, si1):
            t = chunks[j // CH]
            o = (j % CH) * W
            return t[:, o + si0 : o + si1]

        loaded = 0

        # gpsimd sits out the last 4 pairs so its (expensive) SWDGE teardown
        # drain overlaps the matmul stream instead of trailing the kernel
        def flushq(p_):
            if p_ >= HS // 2 - 12:
                return (nc.sync, nc.scalar)[p_ % 2]
            return (nc.gpsimd, nc.sync, nc.scalar)[p_ % 3]

        def emit_dual_head(xr0, p0, xr1, p1):
            # head-output partition layout (channel-row-interleaved):
            #   base+2q+e   = head channel q (cls/box/dir), row-parity e
            #   base+32+2s+e = scr channel s (sigmoid; base+32 is 32-aligned
            #   for ACT), so S[base:base+40] maps 1:1 onto the pair's
            #   [20, 2, 512] output block -> single-descriptor flush
            S = opool.tile([128, 2 * W], f32, tag="S", name="S")
            hP = hpool.tile([128, W], f32, tag="hp")
            nc.tensor.matmul(hP[0:40, :], whd[:], xr0[:], start=True, stop=True)
            nc.tensor.matmul(hP[64:104, :], whd[:], xr1[:], start=True, stop=True)
            for base, col, p_ in ((0, 0, p0), (64, W, p1)):
                cs = slice(col, col + W)
                nc.vector.tensor_scalar_add(
                    S[base : base + 32, cs], hP[base : base + 32, :], bt2[base : base + 32, :]
                )
                nc.scalar.activation(
                    S[base + 32 : base + 40, cs],
                    hP[base + 32 : base + 40, :],
                    Sigmoid,
                    bias=bt2[base + 32 : base + 40, :],
                )
                flushq(p_).dma_start(
                    out_d[:, 2 * p_ * W : (2 * p_ + 2) * W], S[base : base + 40, cs]
                )

        def conv_fulls_kx(p, P, kx, first):
            # each logical full matmul = two concurrent col-tiled M=64 twins
            # sharing one moving stream, so every slot in the kernel runs in
            # (128,64) tile mode and the array never pays a mode-switch drain
            b, c = 2 * p + 1, 2 * p + 2
            si0, si1, so0, so1 = CUTS[kx]
            for t_idx, j in ((0, b), (1, c)):
                w = wall[:, FULLS[kx] + t_idx * 128 : FULLS[kx] + (t_idx + 1) * 128]
                rs = row_slice(j, si0, si1)
                nc.tensor.matmul(
                    P[0:64, so0:so1], w[:, 0:64], rs, start=first, stop=False
                )
                nc.tensor.matmul(
                    P[64:128, so0:so1], w[:, 64:128], rs, start=first, stop=False
                )
                first = False

        def conv_halves_kx(p, P, kx):
            a, d = 2 * p, 2 * p + 3
            si0, si1, so0, so1 = CUTS[kx]
            last = kx == 2
            hb = HALVES[kx]
            nc.tensor.matmul(
                P[0:64, so0:so1],
                wall[:, hb : hb + 64],
                row_slice(a, si0, si1),
                start=False,
                stop=last,
            )
            nc.tensor.matmul(
                P[64:128, so0:so1],
                wall[:, hb + 64 : hb + 128],
                row_slice(d, si0, si1),
                start=False,
                stop=last,
            )

        pend = []  # [(xr, pair_idx), ...] heads not yet emitted
        for q in range(HS // 4):  # 2-pair groups
            p0, p1 = 2 * q, 2 * q + 1
            cneed = (2 * p1 + 3) // CH
            while loaded <= min(cneed + 3, NCHUNK - 1):
                load_chunk(loaded)
                loaded += 1

            P0 = ppool.tile([128, W], f32, tag="pp")
            P1 = ppool.tile([128, W], f32, tag="pp")
            if q == 0:
                # kx-major order for the first group: 6 slots of kx=1-only
                # work cover the in-flight kx=0/kx=2 weight descriptors;
                # fulls before halves keeps the needed x rows ascending
                for kx in (1, 0, 2):
                    conv_fulls_kx(p0, P0, kx, first=kx == 1)
                    conv_fulls_kx(p1, P1, kx, first=kx == 1)
                    conv_halves_kx(p0, P0, kx)
                    conv_halves_kx(p1, P1, kx)
                xr0 = xrpool.tile([128, W], bf16, tag="xr")
                nc.vector.tensor_scalar_max(xr0[:], P0[:], 0.0)
                xr1 = xrpool.tile([128, W], bf16, tag="xr")
                nc.vector.tensor_scalar_max(xr1[:], P1[:], 0.0)
            else:
                for kx in (1, 0, 2):
                    conv_fulls_kx(p0, P0, kx, first=kx == 1)
                for kx in (1, 0, 2):
                    conv_fulls_kx(p1, P1, kx, first=kx == 1)
                for kx in (1, 0, 2):
                    conv_halves_kx(p0, P0, kx)
                xr0 = xrpool.tile([128, W], bf16, tag="xr")
                nc.vector.tensor_scalar_max(xr0[:], P0[:], 0.0)
                for kx in (1, 0, 2):
                    conv_halves_kx(p1, P1, kx)
                xr1 = xrpool.tile([128, W], bf16, tag="xr")
                nc.vector.tensor_scalar_max(xr1[:], P1[:], 0.0)

            if pend:
                emit_dual_head(pend[0][0], pend[0][1], pend[1][0], pend[1][1])
            pend = [(xr0, p0), (xr1, p1)]

        emit_dual_head(pend[0][0], pend[0][1], pend[1][0], pend[1][1])

    nc.compile()
    return nc


def _get_nc():
    if "nc" not in _NC_CACHE:
        _NC_CACHE["nc"] = _build_nc()
    return _NC_CACHE["nc"]


def _pack_weights(w_shared, w_cls, b_cls, w_box, b_box, w_dir, b_dir, w_scr, b_scr):
    # fold the fp8 x pre-scale into the conv weights
    Wt = (np.ascontiguousarray(w_shared, np.float32) / XSCALE).transpose(1, 0, 2, 3)
    wall = np.zeros((128, WALLC), np.float32)
    for kx in range(3):
        f0 = FULLS[kx]
        wall[:, f0 + 0 : f0 + 64] = Wt[:, :, 1, kx]
        wall[:, f0 + 64 : f0 + 128] = Wt[:, :, 0, kx]
        wall[:, f0 + 128 : f0 + 192] = Wt[:, :, 2, kx]
        wall[:, f0 + 192 : f0 + 256] = Wt[:, :, 1, kx]
        h0 = HALVES[kx]
        wall[:, h0 + 0 : h0 + 64] = Wt[:, :, 0, kx]
        wall[:, h0 + 64 : h0 + 128] = Wt[:, :, 2, kx]

    Wh = np.concatenate([w_cls, w_box, w_dir, w_scr], 0)[:, :, 0, 0].astype(np.float32)  # [20,64]
    bh = np.concatenate([b_cls, b_box, b_dir, b_scr], 0).astype(np.float32)  # [20]
    # channel-row-interleaved head layout: partition 2q+e = channel q of
    # row-parity e (k-half e holds that row's 64 conv channels); scr channels
    # at 32+2s+e so the sigmoid rows start 32-aligned for ACT
    for qq in range(16):
        wall[0:64, WHD0 + 2 * qq] = Wh[qq]
        wall[64:128, WHD0 + 2 * qq + 1] = Wh[qq]
    for s in range(4):
        wall[0:64, WHD0 + 32 + 2 * s] = Wh[16 + s]
        wall[64:128, WHD0 + 33 + 2 * s] = Wh[16 + s]
    import ml_dtypes

    wall = np.ascontiguousarray(wall).astype(ml_dtypes.bfloat16)
    b40 = np.empty((40,), np.float32)
    b40[0:32] = np.repeat(bh[0:16], 2)
    b40[32:40] = np.repeat(bh[16:20], 2)
    b40 = np.ascontiguousarray(b40[:, None])  # [40,1]
    return wall, b40


def _make_in_maps(inputs):
    import ml_dtypes

    feature = np.ascontiguousarray(inputs["feature"], np.float32)  # [4,128,512,512]
    B, Cin, H, Wd = feature.shape
    assert (B, Cin, H, Wd) == (4, 128, 512, 512)

    wall, b40 = _pack_weights(
        np.asarray(inputs["w_shared"]),
        np.asarray(inputs["w_cls"]), np.asarray(inputs["b_cls"]),
        np.asarray(inputs["w_box"]), np.asarray(inputs["b_box"]),
        np.asarray(inputs["w_dir"]), np.asarray(inputs["b_dir"]),
        np.asarray(inputs["w_scr"]), np.asarray(inputs["b_scr"]),
    )

    f8 = np.clip(feature * XSCALE, -15.5, 15.5).astype(ml_dtypes.float8_e3m4)

    in_maps = []
    for core in range(8):
        bi, half = core // 2, core % 2
        r0 = half * HS
        xs = np.zeros((128, HALO, W), ml_dtypes.float8_e3m4)
        lo, hi = r0 - 1, r0 + HS + 1
        slo, shi = max(lo, 0), min(hi, H)
        xs[:, slo - lo : HALO - (hi - shi), :] = f8[bi, :, slo:shi, :]
        in_maps.append(
            {
                "x": xs.reshape(128, HALO * W),
                "wall": wall,
                "b40": b40,
            }
        )
    return in_maps


def _gather(res):
    out = np.empty((4, 20, 512, 512), np.float32)
    for core in range(8):
        bi, half = core // 2, core % 2
        out[bi, :, half * HS : (half + 1) * HS, :] = res.results[core]["out"].reshape(
            20, HS, W
        )
    return out


def kernel(**inputs):
    from concourse.bass_utils import run_bass_kernel_spmd

    in_maps = _make_in_maps(inputs)
    nc = _get_nc()
    res = run_bass_kernel_spmd(nc, in_maps, core_ids=list(range(8)))
    return _gather(res)


def run_traced(**inputs):
    """Like kernel(), but returns (out, BassKernelResults) with a profile trace."""
    from concourse.bass_utils import run_bass_kernel_spmd

    in_maps = _make_in_maps(inputs)
    nc = _get_nc()
    res = run_bass_kernel_spmd(nc, in_maps, core_ids=list(range(8)), trace=True)
    return _gather(res), res
